# revision 13
# baseline (speedup 1.0000x reference)
"""Trainium2 Bass kernel for the k-mer transformer problem.

Semantics (k=3, one-hot 3-mer filters over 4 bases):
    z[b, c, l] = relu(x[b,0,l,d0] + x[b,0,l+1,d1] + x[b,0,l+2,d2] - 2)
      where c = 16*d0 + 4*d1 + d2,  l in [0, 99999)
    out[b, 0, r*33333 + q, c] = z[b, c, 3q + r]      (mod-3 interleave)

Strategy: pure data parallel (batch elem b -> NeuronCore b). Per core the
output (25.6 MB) is produced directly in the permuted order so every store
is a dense contiguous DMA. The pipeline is organized so the store DMAs
(the ~76 us HBM-write roofline) stream continuously from ~6 us on:

  - g-outer/r-inner loop: one store DMA per g-chunk covers all 3 phases
    (y is laid out [P, 3, QP*64] so the SBUF->HBM AP orders match), which
    keeps the HWDGE DMA count at 8 (== the 8 DMAHW sem lanes).
  - input is loaded in 4 pieces via SWDGE (gpsimd) so compute starts after
    ~0.2 MB instead of the full 1.6 MB, and loads don't consume HWDGE lanes.
  - channel expansion (two broadcast-AP tensor_tensor adds) on DVE;
    relu(x-2) on ACT (r=0,1, fused bias+Relu) and Pool (r=2, tensor_scalar
    sub-2/max-0) so no single engine exceeds the store roofline.
  - chunk sizes warm up small (fast first store) and taper at the end
    (small last store -> short tail).

Per-partition layout: partition p owns q in [261*p, 261*(p+1)), i.e. x rows
[783*p, 783*p + 785). The host stages x as a [128, 3160] f32 array whose
row p is x.flat[3132*p : 3132*p + 3160] (zero padded past the end).
"""

import sys

import numpy as np

sys.path.insert(0, "/opt/trn_rl_repo")

import concourse.bacc as bacc  # noqa: E402
import concourse.mybir as mybir  # noqa: E402
from concourse.bass_utils import run_bass_kernel_spmd  # noqa: E402
from concourse.tile import TileContext  # noqa: E402

P = 128  # SBUF partitions
QP = 261  # q-positions per partition (padded: 128*261 = 33408 >= 33333)
Q = 33333  # valid q-positions per phase (99999 / 3)
# g-chunk sizes (sum 261): warm-up small for an early first store, taper at
# the end so the final store (and its exposed tail) is small.
CHUNKS = [6, 10, 16, 24, 32, 40, 48, 45, 28, 12]
MAXG = max(CHUNKS)
# input pieces: chunk-index spans; piece i covers x cols [12*gs, 12*ge + 16)
PIECES = [(0, 1), (1, 3), (3, 6), (6, 10)]
XW = 3160  # staged f32 per partition
XSTRIDE = 3132  # f32 advance per partition (783 rows * 4 ch)
L = 100001
N_CORES = 8

_CACHE = {}


def _build_bass():
    nc = bacc.Bacc()
    f32 = mybir.dt.float32
    add = mybir.AluOpType.add
    sub = mybir.AluOpType.subtract
    amax = mybir.AluOpType.max
    relu = mybir.ActivationFunctionType.Relu

    x_d = nc.declare_dram_parameter("x", [P, XW], f32, isOutput=False)
    y_d = nc.declare_dram_parameter("y", [P, 3, QP * 64], f32, isOutput=True)

    # chunk start offsets
    starts = []
    g = 0
    for G in CHUNKS:
        starts.append(g)
        g += G
    assert g == QP

    # piece column ranges and chunk -> piece map
    piece_cols = []
    chunk_piece = {}
    for i, (sc, ec) in enumerate(PIECES):
        gs, ge = starts[sc], (starts[ec] if ec < len(CHUNKS) else QP)
        piece_cols.append((12 * gs, min(12 * ge + 16, XW)))
        for c in range(sc, ec):
            chunk_piece[c] = i

    with TileContext(nc) as tc:
        with (
            tc.tile_pool(name="xp", bufs=1) as xp,
            tc.tile_pool(name="t1p", bufs=2) as t1p,
            tc.tile_pool(name="t2p", bufs=3) as t2p,
            tc.tile_pool(name="op_", bufs=4) as op_,
        ):
            bias_sb = xp.tile([P, 1], f32, tag="bias")
            nc.vector.memset(bias_sb, -2.0)
            # input pieces: SWDGE loads (gpsimd) keep the 8 HWDGE sem lanes
            # free for the 8 stores
            px = []
            for i, (c0, c1) in enumerate(piece_cols):
                t = xp.tile([P, c1 - c0], f32, tag=f"px{i}")
                nc.sync.dma_start(out=t, in_=x_d[:, c0:c1])
                px.append(t)

            for c, G in enumerate(CHUNKS):
                g0 = starts[c]
                pt = px[chunk_piece[c]]
                pbase = piece_cols[chunk_piece[c]][0]
                for r in range(3):
                    bl = 12 * g0 + 4 * r - pbase
                    # A[p, t, d0] broadcast over d1: [[12,G],[1,4],[0,4]]
                    a_ap = (
                        pt[:, bl : bl + 12 * G]
                        .rearrange("p (t u) -> p t u", u=12)[:, :, 0:4]
                        .broadcast_to([P, G, 4, 4])
                    )
                    # B[p, t, d1] tiled over d0: [[12,G],[0,4],[1,4]]
                    b_ap = (
                        pt[:, bl + 4 : bl + 4 + 12 * G]
                        .rearrange("p (t u) -> p t u", u=12)[:, :, 0:4]
                        .unsqueeze(2)
                        .broadcast_to([P, G, 4, 4])
                    )
                    t1 = t1p.tile([P, G * 16], f32, tag="t1")
                    nc.vector.tensor_tensor(
                        t1.rearrange("p (t a b) -> p t a b", a=4, b=4),
                        a_ap,
                        b_ap,
                        add,
                    )
                    # T1[p, t, e] broadcast over d2: [[16,G],[1,16],[0,4]]
                    t1_b = t1.rearrange("p (t e) -> p t e", e=16).broadcast_to(
                        [P, G, 16, 4]
                    )
                    # C[p, t, d2] tiled over e: [[12,G],[0,16],[1,4]]
                    c_ap = (
                        pt[:, bl + 8 : bl + 8 + 12 * G]
                        .rearrange("p (t u) -> p t u", u=12)[:, :, 0:4]
                        .unsqueeze(2)
                        .broadcast_to([P, G, 16, 4])
                    )
                    t2 = t2p.tile([P, G * 64], f32, tag="t2")
                    nc.vector.tensor_tensor(
                        t2.rearrange("p (t e b) -> p t e b", e=16, b=4),
                        t1_b,
                        c_ap,
                        add,
                    )
                    # relu(t2 - 2) fused on the scalar engine (Pool's Q7
                    # tensor_scalar measured ~10 ns/elem -- unusable)
                    o = op_.tile([P, G * 64], f32, tag="o")
                    nc.scalar.activation(o, t2, relu, bias=bias_sb)
                    # per-(chunk, phase) store: releases bytes to the HBM
                    # stream as soon as each phase is relu'd. All DMA issues
                    # live on the otherwise-idle Sync engine (one HWDGE ring
                    # alone saturates the 16 SDMA engines).
                    nc.sync.dma_start(
                        out=y_d[:, r, g0 * 64 : (g0 + G) * 64], in_=o
                    )
    return nc


def _stage_inputs(x):
    """x: [8, 1, L, 4] f32 -> list of per-core {'x': [P, XW] f32}."""
    need = XSTRIDE * (P - 1) + XW
    in_maps = []
    for b in range(x.shape[0]):
        xf = np.zeros(need, dtype=np.float32)
        xf[: L * 4] = x[b, 0].ravel()
        xs = np.lib.stride_tricks.as_strided(
            xf, shape=(P, XW), strides=(XSTRIDE * 4, 4)
        )
        in_maps.append({"x": np.ascontiguousarray(xs)})
    return in_maps


def _gather_output(results):
    out = np.empty((len(results), 1, 3 * Q, 64), dtype=np.float32)
    for b, res in enumerate(results):
        y = res["y"].reshape(P, 3, QP, 64)
        y = y.transpose(1, 0, 2, 3).reshape(3, P * QP, 64)[:, :Q, :]
        out[b, 0] = y.reshape(3 * Q, 64)
    return out


def _built_and_finalized():
    if "nc" not in _CACHE:
        nc = _build_bass()
        nc.finalize()
        _CACHE["nc"] = nc
    return _CACHE["nc"]


def run(x, trace=False):
    nc = _built_and_finalized()
    in_maps = _stage_inputs(np.asarray(x, dtype=np.float32))
    bkr = run_bass_kernel_spmd(nc, in_maps, list(range(N_CORES)), trace=trace)
    return _gather_output(bkr.results), bkr


def kernel(x, W=None):
    out, _ = run(x, trace=False)
    return out
